# revision 41
# baseline (speedup 1.0000x reference)
"""GQA attention (RoPE, causal) + o_proj on 8 Trainium2 NeuronCores.

Sharding: 8 cores = 2 batches (DP) x 4 head-groups (TP over GQA groups).
Per core: hsT[batch] [D,S] (host-pretransposed fp16), Wq slice [D,512]
(8 q heads), Wk/Wv slice [D,128] (2 kv heads), Wo slice [512,D]. Core
computes its heads' attention and a partial o_proj output [S,D] fp16;
host sums 4 partials per batch in fp32.

Kernel (per core; fp16 matmul operands, fp32 PSUM accumulate). The
Trainium2 PE downclocks 2.4 -> 1.2 GHz whenever its pipeline gaps and
needs ~3us of continuous work to ramp back, so the whole kernel is one
fused software-pipelined loop over 4 sequence supertiles J (512
positions each) arranged to keep the PE stream dependency-free:

  proj units: q/k/v projections computed PRE-TRANSPOSED via lhsT =
      weight-chunk (out [features, s-cols], 512-col matmul chains; no
      PE transposes for q/k, no short kv matmuls). RoPE applied in the
      transposed layout on DVE in fp16 (4x mode) with partition-shifted
      rotate-half copies against [128,S] fp16 sign-folded trig tables;
      the final RoPE add writes qT directly. k replicated to both
      64-partition halves of kT2 (gpsimd copies) for head-pair-aligned
      score matmuls; v transposed back to natural via 4 PE transposes
      per block into vaug (with a ones column for the denominator).
  attn(J,t): per head pair: scores^T[k,q] = kT2.T @ qT per 128-k-tile
      with 128-granular causal trim on the 4 diagonal k-tiles (trimmed
      chunks packed contiguously in PSUM so one exp per k-tile pair
      covers exactly the needed elements); exp on ACT (the only ACT
      work in attention sections) -> fp16 P^T; corner triangle masked
      by DVE multiply; PV interleaved one k-pair behind scores;
      A^T_aug = [V|1].T @ P^T in PSUM, row 64 = softmax denominator;
      normalize via fast reciprocal + gpsimd partition_broadcast + DVE
      multiply into aT.
  schedule: proj units for block J+1 and o_proj s-tile chunks for block
      J-1 are interleaved between attn(J,t) blocks, keeping the PE busy
      while ACT drains exp and DVE/gpsimd run RoPE/normalize.
  DMA: inputs are fetched on four different engine queues in parallel
      (sync/vector/scalar/gpsimd) so the lead-in is not serialized.

PSUM plan (8 banks exactly, slots time-shared via pool tag rings):
  ps_big  2x[128,1024]f32 (4 banks): q-proj psum, score tiles
  ps_sky  2x[128, 512]f32 (2 banks): k/v-proj psum, o_proj psum
  ps_ax   2x[128, 512]f32 (2 banks): v-transposes, A^T accumulators
"""
import sys
import numpy as np

sys.path.insert(0, "/opt/trn_rl_repo")

B, S, D = 2, 2048, 2048
H, KVH, HD = 32, 8, 64
SCALE = HD ** -0.5
P = 128


def build_nc(S=S, D=D, LQ=8, LKV=2, HD=64):
    import concourse.bacc as bacc
    import concourse.mybir as mybir
    from concourse import tile
    from concourse.masks import make_identity

    f32 = mybir.dt.float32
    f16 = mybir.dt.float16

    QF = LQ * HD          # local q features (512)
    KF = LKV * HD         # local kv features (128)
    FT = QF // P          # q feature chunks = head pairs (4)
    DT = D // P           # contraction tiles (16)
    ST = S // P           # sequence tiles (16)
    NJ = S // 512         # q supertiles (4)
    VW = HD + 1           # v + ones column (65)
    Exp = mybir.ActivationFunctionType.Exp

    nc = bacc.Bacc(None, target_bir_lowering=False)
    # all inputs host-pre-tiled to [128, ...] partition-major contiguous
    # layouts so every DMA moves multi-KB contiguous lines per partition
    hsT4 = nc.declare_dram_parameter("hsT4", [P, S // 512, DT, 512], f16,
                                     isOutput=False)
    wq_t = nc.declare_dram_parameter("wqt", [P, DT, QF], f16, isOutput=False)
    wkv_t = nc.declare_dram_parameter("wkvt", [P, DT, 2 * KF], f16,
                                      isOutput=False)
    wo_t = nc.declare_dram_parameter("wot", [P, FT, D], f16, isOutput=False)
    cqT = nc.declare_dram_parameter("cqT", [P, S], f16, isOutput=False)
    sqT = nc.declare_dram_parameter("sqT", [P, S], f16, isOutput=False)
    ckT = nc.declare_dram_parameter("ckT", [P, S], f16, isOutput=False)
    skT = nc.declare_dram_parameter("skT", [P, S], f16, isOutput=False)
    out = nc.declare_dram_parameter("out", [S, D], f16, isOutput=True)

    out_t = out.rearrange("(st p) d -> p st d", p=P)

    with tile.TileContext(nc) as tc:
        with tc.tile_pool(name="persist", bufs=1) as persist:
            ident = persist.tile([P, P], f16)
            maskc = persist.tile([P, P], f16)
            qT = persist.tile([P, FT, S], f16)
            kT2 = persist.tile([P, LKV, S], f16)
            vaug = persist.tile([P, ST, LKV * VW], f16)
            cq_sb = persist.tile([P, S], f16)
            sq_sb = persist.tile([P, S], f16)
            ck_sb = persist.tile([P, S], f16)
            sk_sb = persist.tile([P, S], f16)
            wq_sb = persist.tile([P, DT, QF], f16)
            wkv_sb = persist.tile([P, DT, 2 * KF], f16)
            wo_sb = persist.tile([P, FT, D], f16)


            with (
                tc.tile_pool(name="hsT", bufs=2) as hsT_p,
                tc.tile_pool(name="st16", bufs=3) as st16_p,
                tc.tile_pool(name="rope", bufs=2) as rope_p,
                tc.tile_pool(name="pt_p", bufs=6) as pt_p,
                tc.tile_pool(name="aT_p", bufs=2) as aT_p,
                tc.tile_pool(name="nrm", bufs=4) as nrm_p,
                tc.tile_pool(name="bc_p", bufs=4) as bc_p,
                tc.tile_pool(name="y_p", bufs=3) as y_p,
                tc.tile_pool(name="ps_big", bufs=2, space="PSUM") as ps_big,
                tc.tile_pool(name="ps_sky", bufs=2, space="PSUM") as ps_sky,
                tc.tile_pool(name="ps_ax", bufs=2, space="PSUM") as ps_ax,
            ):
                hsT_tiles = {}

                def fetch_hsT(j, eng=None, split=False):
                    if j >= NJ or j in hsT_tiles:
                        return
                    t_ = hsT_p.tile([P, DT, 512], f16, tag="hsT",
                                    name=f"hsT{j}")
                    if split:
                        for dg in range(4):
                            (eng or nc.sync).dma_start(
                                out=t_[:, 4 * dg:4 * dg + 4, :],
                                in_=hsT4[:, j, 4 * dg:4 * dg + 4, :])
                    else:
                        (eng or nc.sync).dma_start(
                            out=t_[:], in_=hsT4[:, j, :, :])
                    hsT_tiles[j] = t_

                def rope16T(x16, ctab, stab, dst):
                    # transposed layout [feature-part, s-cols]; heads are
                    # 64-partition groups; rotate-half partners are +-32
                    # partitions within each head (uniform-shift copies)
                    rsh = rope_p.tile([P, 512], f16, tag="rsh")
                    for o in (0, HD):
                        nc.vector.tensor_copy(
                            rsh[o:o + 32, :], x16[o + 32:o + 64, :])
                        nc.vector.tensor_copy(
                            rsh[o + 32:o + 64, :], x16[o:o + 32, :])
                    tmp = rope_p.tile([P, 512], f16, tag="tmp")
                    nc.vector.tensor_mul(tmp[:], x16[:], ctab)
                    nc.vector.tensor_mul(rsh[:], rsh[:], stab)
                    nc.vector.tensor_add(dst, tmp[:], rsh[:])

                def unit_q(j, c):
                    hsT_b = hsT_tiles[j]
                    cols = slice(j * 512, (j + 1) * 512)
                    psq = ps_big.tile([P, 1024], f32, tag="big")
                    for dt in range(DT):
                        nc.tensor.matmul(
                            psq[:, 0:512],
                            wq_sb[:, dt, c * P:(c + 1) * P],
                            hsT_b[:, dt, :],
                            start=(dt == 0), stop=(dt == DT - 1))
                    q16 = st16_p.tile([P, 512], f16, tag="x16")
                    nc.scalar.copy(q16[:], psq[:, 0:512])
                    rope16T(q16, cq_sb[:, cols], sq_sb[:, cols],
                            qT[:, c, cols])

                def unit_k(j):
                    hsT_b = hsT_tiles[j]
                    cols = slice(j * 512, (j + 1) * 512)
                    psk = ps_sky.tile([P, 512], f32, tag="sky")
                    for dt in range(DT):
                        nc.tensor.matmul(
                            psk[:], wkv_sb[:, dt, 0:KF], hsT_b[:, dt, :],
                            start=(dt == 0), stop=(dt == DT - 1))
                    k16 = st16_p.tile([P, 512], f16, tag="x16")
                    nc.scalar.copy(k16[:], psk[:])
                    krot = rope_p.tile([P, 512], f16, tag="krot")
                    rope16T(k16, ck_sb[:, cols], sk_sb[:, cols], krot[:])
                    for g in range(LKV):
                        ksl = krot[g * HD:(g + 1) * HD, :]
                        nc.vector.tensor_copy(kT2[0:HD, g, cols], ksl)
                        nc.vector.tensor_copy(kT2[HD:P, g, cols], ksl)

                def unit_v(j):
                    hsT_b = hsT_tiles[j]
                    cols = slice(j * 512, (j + 1) * 512)
                    psv = ps_sky.tile([P, 512], f32, tag="sky")
                    for dt in range(DT):
                        nc.tensor.matmul(
                            psv[:], wkv_sb[:, dt, KF:2 * KF], hsT_b[:, dt, :],
                            start=(dt == 0), stop=(dt == DT - 1))
                    v16 = st16_p.tile([P, 512], f16, tag="x16")
                    nc.scalar.copy(v16[:], psv[:])
                    vtr = ps_ax.tile([P, 512], f16, tag="ax")
                    for si in range(4):
                        nc.tensor.transpose(
                            vtr[:, si * P:(si + 1) * P],
                            v16[:, si * P:(si + 1) * P], ident[:])
                    for si in range(4):
                        st_i = 4 * j + si
                        for g in range(LKV):
                            nc.vector.tensor_copy(
                                vaug[:, st_i, g * VW:g * VW + HD],
                                vtr[:, si * P + g * HD:si * P + (g + 1) * HD])

                def attn_block(J, t, aTt, midfill=None, fine_norm=False):
                    g = t // 2        # kv head for this q-head pair
                    nkt = 4 * J + 4
                    psa0 = ps_ax.tile([P, 512], f32, tag="ax")
                    psa1 = ps_ax.tile([P, 512], f32, tag="ax")
                    prev = None       # (pt0, pt1, cw) awaiting PV

                    def pv(item):
                        pt0, pt1, cw = item
                        for (kt, cc, w, qo) in cw:
                            va = vaug[:, kt, g * VW:(g + 1) * VW]
                            nc.tensor.matmul(
                                psa0[0:VW, qo:512], va, pt0[:, cc:cc + w],
                                start=(kt == 0), stop=(kt == nkt - 1),
                                skip_group_check=True)
                            nc.tensor.matmul(
                                psa1[0:VW, qo:512], va, pt1[:, cc:cc + w],
                                start=(kt == 0), stop=(kt == nkt - 1),
                                skip_group_check=True)

                    for kp in range(nkt // 2):
                        cw = []
                        c = 0
                        for i in range(2):
                            kt = 2 * kp + i
                            m = max(0, kt - 4 * J)
                            w = 512 - P * m
                            cw.append((kt, c, w, P * m))
                            c += w
                        pss0 = ps_big.tile([P, 1024], f32, tag="big")
                        pss1 = ps_big.tile([P, 1024], f32, tag="big")
                        for (kt, cc, w, qo) in cw:
                            qcol = slice(J * 512 + qo, (J + 1) * 512)
                            nc.tensor.matmul(
                                pss0[:, cc:cc + w],
                                kT2[0:HD, g, kt * P:(kt + 1) * P],
                                qT[0:HD, t, qcol],
                                start=True, stop=True)
                        for (kt, cc, w, qo) in cw:
                            qcol = slice(J * 512 + qo, (J + 1) * 512)
                            nc.tensor.matmul(
                                pss1[:, cc:cc + w],
                                kT2[HD:P, g, kt * P:(kt + 1) * P],
                                qT[HD:P, t, qcol],
                                start=True, stop=True)
                        pt0 = pt_p.tile([P, 1024], f16, tag="pt")
                        pt1 = pt_p.tile([P, 1024], f16, tag="pt")
                        nc.scalar.activation(pt0[:, 0:c], pss0[:, 0:c], Exp)
                        nc.scalar.activation(pt1[:, 0:c], pss1[:, 0:c], Exp)
                        for (kt, cc, w, qo) in cw:
                            if kt >= 4 * J:
                                nc.vector.tensor_mul(
                                    pt0[:, cc:cc + P], pt0[:, cc:cc + P],
                                    maskc[:])
                                nc.vector.tensor_mul(
                                    pt1[:, cc:cc + P], pt1[:, cc:cc + P],
                                    maskc[:])
                        if prev is not None:
                            pv(prev)
                        prev = (pt0, pt1, cw)
                        if midfill is not None and kp == nkt // 4:
                            midfill()
                            midfill = None
                    pv(prev)
                    if midfill is not None:
                        midfill()
                    for psa, poff in ((psa0, 0), (psa1, HD)):
                        dn = nrm_p.tile([1, 512], f32, tag="dn")
                        nc.vector.tensor_copy(dn[:], psa[HD:VW, :])
                        rc = nrm_p.tile([1, 512], f32, tag="rc")
                        nc.vector.reciprocal_approx_fast(rc[:], dn[:])
                        dnb = bc_p.tile([HD, 512], f32, tag="bc")
                        nc.gpsimd.partition_broadcast(dnb[:], rc[:])
                        if fine_norm:
                            # last block: 128-col pieces so the tail o_proj
                            # LDWs unblock as soon as their slice is done
                            for sl in range(4):
                                c0 = sl * P
                                nc.vector.tensor_mul(
                                    aTt[poff:poff + HD, t, c0:c0 + P],
                                    psa[0:HD, c0:c0 + P], dnb[:, c0:c0 + P])
                        else:
                            nc.vector.tensor_mul(
                                aTt[poff:poff + HD, t, :], psa[0:HD, :],
                                dnb[:])

                def oproj_chunk(aTt, st, tail=False):
                    stl = st % 4
                    for dn_i in range(D // 512):
                        psy = ps_sky.tile([P, 512], f32, tag="sky")
                        for ft in range(FT):
                            nc.tensor.matmul(
                                psy[:],
                                aTt[:, ft, stl * P:(stl + 1) * P],
                                wo_sb[:, ft, dn_i * 512:(dn_i + 1) * 512],
                                start=(ft == 0), stop=(ft == FT - 1))
                        yt = y_p.tile([P, 512], f16, tag="yt")
                        if tail and dn_i % 2 == 1:
                            nc.scalar.copy(yt[:], psy[:])
                        else:
                            nc.vector.tensor_copy(yt[:], psy[:])
                        deng = (nc.sync, nc.scalar, nc.gpsimd,
                                nc.sync)[dn_i] if tail else nc.sync
                        deng.dma_start(
                            out=out_t[:, st, dn_i * 512:(dn_i + 1) * 512],
                            in_=yt[:])

                # prologue: input DMAs spread across the three
                # DMA-capable queues (sync/scalar/gpsimd), first-needed
                # first; hsT block 0 is quarter-split across two queues
                t0_ = hsT_p.tile([P, DT, 512], f16, tag="hsT", name="hsT0")
                hsT_tiles[0] = t0_
                # interleave hsT0 pieces and wkv quarters across all three
                # DMA queues in the order the unit_k dt-chain consumes them
                q3 = [nc.gpsimd, nc.sync, nc.scalar]
                plan = [("w", 0), ("h", 0), ("h", 1), ("w", 1), ("h", 2),
                        ("h", 3), ("w", 2), ("h", 4), ("h", 5), ("w", 3),
                        ("h", 6), ("h", 7)]
                for i, (kind, idx) in enumerate(plan):
                    eng = q3[i % 3]
                    if kind == "h":
                        eng.dma_start(
                            out=t0_[:, 2 * idx:2 * idx + 2, :],
                            in_=hsT4[:, 0, 2 * idx:2 * idx + 2, :])
                    else:
                        eng.dma_start(
                            out=wkv_sb[:, 4 * idx:4 * idx + 4, :],
                            in_=wkv_t[:, 4 * idx:4 * idx + 4, :])
                for dq in range(4):
                    eng = (nc.sync, nc.scalar, nc.gpsimd, nc.sync)[dq]
                    eng.dma_start(out=wq_sb[:, 4 * dq:4 * dq + 4, :],
                                  in_=wq_t[:, 4 * dq:4 * dq + 4, :])
                nc.gpsimd.dma_start(out=ck_sb[:], in_=ckT[:, :])
                nc.scalar.dma_start(out=sk_sb[:], in_=skT[:, :])
                nc.scalar.dma_start(out=cq_sb[:], in_=cqT[:, :])
                nc.gpsimd.dma_start(out=sq_sb[:], in_=sqT[:, :])
                fetch_hsT(1)
                nc.scalar.dma_start(out=wo_sb[:], in_=wo_t[:, :, :])

                # init constants (tiles from main pools: no address-space
                # WAR against the hsT prefetches)
                ident_f = st16_p.tile([P, P], f32, tag="x16")
                make_identity(nc, ident_f[:])
                nc.vector.tensor_copy(ident[:], ident_f[:])
                # corner keep-mask: maskc[kr, qc] = 1 where qc >= kr
                mask_f = st16_p.tile([P, P], f32, tag="x16")
                nc.gpsimd.memset(mask_f[:], 1.0)
                nc.gpsimd.affine_select(
                    out=mask_f[:], in_=mask_f[:],
                    compare_op=mybir.AluOpType.is_ge, fill=0.0,
                    base=0, pattern=[[1, P]], channel_multiplier=-1,
                )
                nc.vector.tensor_copy(maskc[:], mask_f[:])
                for st_i in range(ST):
                    for g in range(LKV):
                        nc.gpsimd.memset(
                            vaug[:, st_i, g * VW + HD:g * VW + HD + 1], 1.0)

                # project block 0 (k/q0/v first so attn(0,0) is ready asap)
                unit_k(0)
                unit_v(0)
                for c in range(FT):
                    unit_q(0, c)

                aT_tiles = {}
                for J in range(NJ):
                    # proj units for J+1 spread across this block's t-slots
                    units = []
                    if J + 1 < NJ:
                        jj = J + 1
                        units = [
                            [lambda: fetch_hsT(jj + 1), lambda: unit_k(jj),
                             lambda: unit_q(jj, 0)],
                            [lambda: unit_v(jj), lambda: unit_q(jj, 1)],
                            [lambda: unit_q(jj, 2)],
                            [lambda: unit_q(jj, 3)],
                        ]
                    aT_tiles[J] = aT_p.tile([P, FT, 512], f16, tag="aT",
                                            name=f"aT{J}")
                    for t in range(FT):
                        mf = None
                        if J >= 1:
                            prev_aT = aT_tiles[J - 1]
                            st_prev = 4 * (J - 1) + t
                            mf = (lambda a=prev_aT, s=st_prev:
                                  oproj_chunk(a, s))
                        attn_block(J, t, aT_tiles[J], midfill=mf,
                                   fine_norm=(J == NJ - 1 and t == FT - 1))
                        if units:
                            for u in units[t]:
                                u()
                    if J >= 2:
                        del aT_tiles[J - 2]
                for t in range(FT):
                    oproj_chunk(aT_tiles[NJ - 1], 4 * (NJ - 1) + t,
                                tail=True)
    nc.compile()
    return nc


def _host_tables(cos, sin, scale):
    # transposed [feature, S] tables, head-pair replicated (2 x 64 rows),
    # rotate-half sign folded into sin rows 0:32 of each 64-row head
    hd = cos.shape[1]                      # 64
    cosT = np.ascontiguousarray(cos.T)     # [64, S]
    sinT = np.ascontiguousarray(sin.T)
    sin_pm = np.concatenate([-sinT[:hd // 2], sinT[hd // 2:]], axis=0)
    cqT = np.tile(cosT * scale, (2, 1)).astype(np.float16)
    sqT = np.tile(sin_pm * scale, (2, 1)).astype(np.float16)
    ckT = np.tile(cosT, (2, 1)).astype(np.float16)
    skT = np.tile(sin_pm, (2, 1)).astype(np.float16)
    return cqT, sqT, ckT, skT


def prepare_in_maps(hidden_states, cos, sin, Wq, Wk, Wv, Wo, LQ=8, LKV=2):
    cos = np.asarray(cos, dtype=np.float32)
    sin = np.asarray(sin, dtype=np.float32)
    cqT, sqT, ckT, skT = _host_tables(cos, sin, SCALE)
    hidden_states = np.asarray(hidden_states, dtype=np.float32)
    Wq = np.asarray(Wq, dtype=np.float32)
    Wk = np.asarray(Wk, dtype=np.float32)
    Wv = np.asarray(Wv, dtype=np.float32)
    Wo = np.asarray(Wo, dtype=np.float32)
    nb = hidden_states.shape[0]
    DT, NJ, FT = 16, 4, 4
    # hsT4[p, j, dt, c] = hs.T[dt*128+p, j*512+c]
    hsT4 = [np.ascontiguousarray(
        hidden_states[b].T.reshape(DT, P, NJ, 512).transpose(1, 2, 0, 3)
    ).astype(np.float16) for b in range(nb)]
    in_maps = []
    for c in range(8):
        b, g2 = c // 4, c % 4
        qs = g2 * LQ * HD
        ks = g2 * LKV * HD
        wq_l = Wq[:, qs:qs + LQ * HD]
        wkv_l = np.concatenate([Wk[:, ks:ks + LKV * HD],
                                Wv[:, ks:ks + LKV * HD]], axis=1)
        wo_l = Wo[qs:qs + LQ * HD, :]
        in_maps.append({
            "hsT4": hsT4[b],
            "wqt": np.ascontiguousarray(
                wq_l.reshape(DT, P, LQ * HD).transpose(1, 0, 2)).astype(np.float16),
            "wkvt": np.ascontiguousarray(
                wkv_l.reshape(DT, P, 2 * LKV * HD).transpose(1, 0, 2)).astype(np.float16),
            "wot": np.ascontiguousarray(
                wo_l.reshape(FT, P, Wo.shape[1]).transpose(1, 0, 2)).astype(np.float16),
            "cqT": cqT, "sqT": sqT, "ckT": ckT, "skT": skT,
        })
    return in_maps


_NC_CACHE = {}


def kernel(hidden_states, attention_mask, cos, sin, Wq, Wk, Wv, Wo):
    from concourse.bass_utils import run_bass_kernel_spmd

    LQ, LKV = 8, 2
    if "nc" not in _NC_CACHE:
        _NC_CACHE["nc"] = build_nc(S, D, LQ, LKV, HD)
    nc = _NC_CACHE["nc"]

    in_maps = prepare_in_maps(hidden_states, cos, sin, Wq, Wk, Wv, Wo, LQ, LKV)
    res = run_bass_kernel_spmd(nc, in_maps, core_ids=list(range(8)))
    y = np.zeros((B, S, D), dtype=np.float32)
    for c in range(8):
        y[c // 4] += res.results[c]["out"].astype(np.float32)
    return y


# revision 42
# speedup vs baseline: 1.0173x; 1.0173x over previous
"""GQA attention (RoPE, causal) + o_proj on 8 Trainium2 NeuronCores.

Sharding: 8 cores = 2 batches (DP) x 4 head-groups (TP over GQA groups).
Per core: hsT[batch] [D,S] (host-pretransposed fp16), Wq slice [D,512]
(8 q heads), Wk/Wv slice [D,128] (2 kv heads), Wo slice [512,D]. Core
computes its heads' attention and a partial o_proj output [S,D] fp16;
host sums 4 partials per batch in fp32.

Kernel (per core; fp16 matmul operands, fp32 PSUM accumulate). The
Trainium2 PE downclocks 2.4 -> 1.2 GHz whenever its pipeline gaps and
needs ~3us of continuous work to ramp back, so the whole kernel is one
fused software-pipelined loop over 4 sequence supertiles J (512
positions each) arranged to keep the PE stream dependency-free:

  proj units: q/k/v projections computed PRE-TRANSPOSED via lhsT =
      weight-chunk (out [features, s-cols], 512-col matmul chains; no
      PE transposes for q/k, no short kv matmuls). RoPE applied in the
      transposed layout on DVE in fp16 (4x mode) with partition-shifted
      rotate-half copies against [128,S] fp16 sign-folded trig tables;
      the final RoPE add writes qT directly. k replicated to both
      64-partition halves of kT2 (gpsimd copies) for head-pair-aligned
      score matmuls; v transposed back to natural via 4 PE transposes
      per block into vaug (with a ones column for the denominator).
  attn(J,t): per head pair: scores^T[k,q] = kT2.T @ qT per 128-k-tile
      with 128-granular causal trim on the 4 diagonal k-tiles (trimmed
      chunks packed contiguously in PSUM so one exp per k-tile pair
      covers exactly the needed elements); exp on ACT (the only ACT
      work in attention sections) -> fp16 P^T; corner triangle masked
      by DVE multiply; PV interleaved one k-pair behind scores;
      A^T_aug = [V|1].T @ P^T in PSUM, row 64 = softmax denominator;
      normalize via fast reciprocal + gpsimd partition_broadcast + DVE
      multiply into aT.
  schedule: proj units for block J+1 and o_proj s-tile chunks for block
      J-1 are interleaved between attn(J,t) blocks, keeping the PE busy
      while ACT drains exp and DVE/gpsimd run RoPE/normalize.
  DMA: inputs are fetched on four different engine queues in parallel
      (sync/vector/scalar/gpsimd) so the lead-in is not serialized.

PSUM plan (8 banks exactly, slots time-shared via pool tag rings):
  ps_big  2x[128,1024]f32 (4 banks): q-proj psum, score tiles
  ps_sky  2x[128, 512]f32 (2 banks): k/v-proj psum, o_proj psum
  ps_ax   2x[128, 512]f32 (2 banks): v-transposes, A^T accumulators
"""
import sys
import numpy as np

sys.path.insert(0, "/opt/trn_rl_repo")

B, S, D = 2, 2048, 2048
H, KVH, HD = 32, 8, 64
SCALE = HD ** -0.5
P = 128


def build_nc(S=S, D=D, LQ=8, LKV=2, HD=64):
    import concourse.bacc as bacc
    import concourse.mybir as mybir
    from concourse import tile
    from concourse.masks import make_identity

    f32 = mybir.dt.float32
    f16 = mybir.dt.float16

    QF = LQ * HD          # local q features (512)
    KF = LKV * HD         # local kv features (128)
    FT = QF // P          # q feature chunks = head pairs (4)
    DT = D // P           # contraction tiles (16)
    ST = S // P           # sequence tiles (16)
    NJ = S // 512         # q supertiles (4)
    VW = HD + 1           # v + ones column (65)
    Exp = mybir.ActivationFunctionType.Exp

    nc = bacc.Bacc(None, target_bir_lowering=False)
    # all inputs host-pre-tiled to [128, ...] partition-major contiguous
    # layouts so every DMA moves multi-KB contiguous lines per partition
    hsT4 = nc.declare_dram_parameter("hsT4", [P, S // 512, DT, 512], f16,
                                     isOutput=False)
    wq_t = nc.declare_dram_parameter("wqt", [P, DT, QF], f16, isOutput=False)
    wkv_t = nc.declare_dram_parameter("wkvt", [P, DT, 2 * KF], f16,
                                      isOutput=False)
    wo_t = nc.declare_dram_parameter("wot", [P, FT, D], f16, isOutput=False)
    cqT = nc.declare_dram_parameter("cqT", [P, S], f16, isOutput=False)
    sqT = nc.declare_dram_parameter("sqT", [P, S], f16, isOutput=False)
    ckT = nc.declare_dram_parameter("ckT", [P, S], f16, isOutput=False)
    skT = nc.declare_dram_parameter("skT", [P, S], f16, isOutput=False)
    out = nc.declare_dram_parameter("out", [S, D], f16, isOutput=True)

    out_t = out.rearrange("(st p) d -> p st d", p=P)

    with tile.TileContext(nc) as tc:
        with tc.tile_pool(name="persist", bufs=1) as persist:
            ident = persist.tile([P, P], f16)
            maskc = persist.tile([P, P], f16)
            qT = persist.tile([P, FT, S], f16)
            kT2 = persist.tile([P, LKV, S], f16)
            vaug = persist.tile([P, ST, LKV * VW], f16)
            cq_sb = persist.tile([P, S], f16)
            sq_sb = persist.tile([P, S], f16)
            ck_sb = persist.tile([P, S], f16)
            sk_sb = persist.tile([P, S], f16)
            wq_sb = persist.tile([P, DT, QF], f16)
            wkv_sb = persist.tile([P, DT, 2 * KF], f16)
            wo_sb = persist.tile([P, FT, D], f16)


            with (
                tc.tile_pool(name="hsT", bufs=2) as hsT_p,
                tc.tile_pool(name="st16", bufs=3) as st16_p,
                tc.tile_pool(name="rope", bufs=2) as rope_p,
                tc.tile_pool(name="pt_p", bufs=6) as pt_p,
                tc.tile_pool(name="aT_p", bufs=2) as aT_p,
                tc.tile_pool(name="nrm", bufs=4) as nrm_p,
                tc.tile_pool(name="bc_p", bufs=4) as bc_p,
                tc.tile_pool(name="y_p", bufs=3) as y_p,
                tc.tile_pool(name="ps_big", bufs=2, space="PSUM") as ps_big,
                tc.tile_pool(name="ps_sky", bufs=2, space="PSUM") as ps_sky,
                tc.tile_pool(name="ps_ax", bufs=2, space="PSUM") as ps_ax,
            ):
                hsT_tiles = {}

                def fetch_hsT(j, eng=None, split=False):
                    if j >= NJ or j in hsT_tiles:
                        return
                    t_ = hsT_p.tile([P, DT, 512], f16, tag="hsT",
                                    name=f"hsT{j}")
                    if split:
                        for dg in range(4):
                            (eng or nc.sync).dma_start(
                                out=t_[:, 4 * dg:4 * dg + 4, :],
                                in_=hsT4[:, j, 4 * dg:4 * dg + 4, :])
                    else:
                        (eng or nc.sync).dma_start(
                            out=t_[:], in_=hsT4[:, j, :, :])
                    hsT_tiles[j] = t_

                def rope16T(x16, ctab, stab, dst):
                    # transposed layout [feature-part, s-cols]; heads are
                    # 64-partition groups; rotate-half partners are +-32
                    # partitions within each head (uniform-shift copies)
                    rsh = rope_p.tile([P, 512], f16, tag="rsh")
                    for o in (0, HD):
                        nc.vector.tensor_copy(
                            rsh[o:o + 32, :], x16[o + 32:o + 64, :])
                        nc.vector.tensor_copy(
                            rsh[o + 32:o + 64, :], x16[o:o + 32, :])
                    tmp = rope_p.tile([P, 512], f16, tag="tmp")
                    nc.vector.tensor_mul(tmp[:], x16[:], ctab)
                    nc.vector.tensor_mul(rsh[:], rsh[:], stab)
                    nc.vector.tensor_add(dst, tmp[:], rsh[:])

                def unit_q(j, c):
                    hsT_b = hsT_tiles[j]
                    cols = slice(j * 512, (j + 1) * 512)
                    psq = ps_big.tile([P, 1024], f32, tag="big")
                    for dt in range(DT):
                        nc.tensor.matmul(
                            psq[:, 0:512],
                            wq_sb[:, dt, c * P:(c + 1) * P],
                            hsT_b[:, dt, :],
                            start=(dt == 0), stop=(dt == DT - 1))
                    q16 = st16_p.tile([P, 512], f16, tag="x16")
                    nc.scalar.copy(q16[:], psq[:, 0:512])
                    rope16T(q16, cq_sb[:, cols], sq_sb[:, cols],
                            qT[:, c, cols])

                def unit_k(j):
                    hsT_b = hsT_tiles[j]
                    cols = slice(j * 512, (j + 1) * 512)
                    psk = ps_sky.tile([P, 512], f32, tag="sky")
                    for dt in range(DT):
                        nc.tensor.matmul(
                            psk[:], wkv_sb[:, dt, 0:KF], hsT_b[:, dt, :],
                            start=(dt == 0), stop=(dt == DT - 1))
                    k16 = st16_p.tile([P, 512], f16, tag="x16")
                    nc.scalar.copy(k16[:], psk[:])
                    krot = rope_p.tile([P, 512], f16, tag="krot")
                    rope16T(k16, ck_sb[:, cols], sk_sb[:, cols], krot[:])
                    for g in range(LKV):
                        ksl = krot[g * HD:(g + 1) * HD, :]
                        nc.vector.tensor_copy(kT2[0:HD, g, cols], ksl)
                        nc.vector.tensor_copy(kT2[HD:P, g, cols], ksl)

                def unit_v(j):
                    hsT_b = hsT_tiles[j]
                    cols = slice(j * 512, (j + 1) * 512)
                    psv = ps_sky.tile([P, 512], f32, tag="sky")
                    for dt in range(DT):
                        nc.tensor.matmul(
                            psv[:], wkv_sb[:, dt, KF:2 * KF], hsT_b[:, dt, :],
                            start=(dt == 0), stop=(dt == DT - 1))
                    v16 = st16_p.tile([P, 512], f16, tag="x16")
                    nc.scalar.copy(v16[:], psv[:])
                    vtr = ps_ax.tile([P, 512], f16, tag="ax")
                    for si in range(4):
                        nc.tensor.transpose(
                            vtr[:, si * P:(si + 1) * P],
                            v16[:, si * P:(si + 1) * P], ident[:])
                    for si in range(4):
                        st_i = 4 * j + si
                        for g in range(LKV):
                            nc.vector.tensor_copy(
                                vaug[:, st_i, g * VW:g * VW + HD],
                                vtr[:, si * P + g * HD:si * P + (g + 1) * HD])

                def attn_block(J, t, aTt, midfill=None, fine_norm=False):
                    g = t // 2        # kv head for this q-head pair
                    nkt = 4 * J + 4
                    psa0 = ps_ax.tile([P, 512], f32, tag="ax")
                    psa1 = ps_ax.tile([P, 512], f32, tag="ax")
                    prev = None       # (pt0, pt1, cw) awaiting PV

                    def pv(item):
                        pt0, pt1, cw = item
                        for (kt, cc, w, qo) in cw:
                            va = vaug[:, kt, g * VW:(g + 1) * VW]
                            nc.tensor.matmul(
                                psa0[0:VW, qo:512], va, pt0[:, cc:cc + w],
                                start=(kt == 0), stop=(kt == nkt - 1),
                                skip_group_check=True)
                            nc.tensor.matmul(
                                psa1[0:VW, qo:512], va, pt1[:, cc:cc + w],
                                start=(kt == 0), stop=(kt == nkt - 1),
                                skip_group_check=True)

                    for kp in range(nkt // 2):
                        cw = []
                        c = 0
                        for i in range(2):
                            kt = 2 * kp + i
                            m = max(0, kt - 4 * J)
                            w = 512 - P * m
                            cw.append((kt, c, w, P * m))
                            c += w
                        pss0 = ps_big.tile([P, 1024], f32, tag="big")
                        pss1 = ps_big.tile([P, 1024], f32, tag="big")
                        for (kt, cc, w, qo) in cw:
                            qcol = slice(J * 512 + qo, (J + 1) * 512)
                            nc.tensor.matmul(
                                pss0[:, cc:cc + w],
                                kT2[0:HD, g, kt * P:(kt + 1) * P],
                                qT[0:HD, t, qcol],
                                start=True, stop=True)
                        for (kt, cc, w, qo) in cw:
                            qcol = slice(J * 512 + qo, (J + 1) * 512)
                            nc.tensor.matmul(
                                pss1[:, cc:cc + w],
                                kT2[HD:P, g, kt * P:(kt + 1) * P],
                                qT[HD:P, t, qcol],
                                start=True, stop=True)
                        pt0 = pt_p.tile([P, 1024], f16, tag="pt")
                        pt1 = pt_p.tile([P, 1024], f16, tag="pt")
                        nc.scalar.activation(pt0[:, 0:c], pss0[:, 0:c], Exp)
                        nc.scalar.activation(pt1[:, 0:c], pss1[:, 0:c], Exp)
                        for (kt, cc, w, qo) in cw:
                            if kt >= 4 * J:
                                nc.vector.tensor_mul(
                                    pt0[:, cc:cc + P], pt0[:, cc:cc + P],
                                    maskc[:])
                                nc.vector.tensor_mul(
                                    pt1[:, cc:cc + P], pt1[:, cc:cc + P],
                                    maskc[:])
                        if prev is not None:
                            pv(prev)
                        prev = (pt0, pt1, cw)
                        if midfill is not None and kp == nkt // 4:
                            midfill()
                            midfill = None
                    pv(prev)
                    if midfill is not None:
                        midfill()
                    for psa, poff in ((psa0, 0), (psa1, HD)):
                        dn = nrm_p.tile([1, 512], f32, tag="dn")
                        nc.vector.tensor_copy(dn[:], psa[HD:VW, :])
                        rc = nrm_p.tile([1, 512], f32, tag="rc")
                        nc.vector.reciprocal_approx_fast(rc[:], dn[:])
                        dnb = bc_p.tile([HD, 512], f32, tag="bc")
                        nc.gpsimd.partition_broadcast(dnb[:], rc[:])
                        if fine_norm:
                            # last block: 128-col pieces so the tail o_proj
                            # LDWs unblock as soon as their slice is done
                            for sl in range(4):
                                c0 = sl * P
                                nc.vector.tensor_mul(
                                    aTt[poff:poff + HD, t, c0:c0 + P],
                                    psa[0:HD, c0:c0 + P], dnb[:, c0:c0 + P])
                        else:
                            nc.vector.tensor_mul(
                                aTt[poff:poff + HD, t, :], psa[0:HD, :],
                                dnb[:])

                def oproj_chunk(aTt, st, tail=False):
                    stl = st % 4
                    for dn_i in range(D // 512):
                        psy = ps_sky.tile([P, 512], f32, tag="sky")
                        for ft in range(FT):
                            nc.tensor.matmul(
                                psy[:],
                                aTt[:, ft, stl * P:(stl + 1) * P],
                                wo_sb[:, ft, dn_i * 512:(dn_i + 1) * 512],
                                start=(ft == 0), stop=(ft == FT - 1))
                        yt = y_p.tile([P, 512], f16, tag="yt")
                        if tail and dn_i % 2 == 1:
                            nc.scalar.copy(yt[:], psy[:])
                        else:
                            nc.vector.tensor_copy(yt[:], psy[:])
                        deng = (nc.sync, nc.scalar, nc.gpsimd,
                                nc.sync)[dn_i] if tail else nc.sync
                        deng.dma_start(
                            out=out_t[:, st, dn_i * 512:(dn_i + 1) * 512],
                            in_=yt[:])

                # prologue: input DMAs spread across the three
                # DMA-capable queues (sync/scalar/gpsimd), first-needed
                # first; hsT block 0 is quarter-split across two queues
                t0_ = hsT_p.tile([P, DT, 512], f16, tag="hsT", name="hsT0")
                hsT_tiles[0] = t0_
                # interleave hsT0 pieces and wkv quarters across all three
                # DMA queues in the order the unit_k dt-chain consumes them
                q3 = [nc.gpsimd, nc.sync, nc.scalar]
                plan = [("w", 0), ("h", 0), ("h", 1), ("w", 1), ("h", 2),
                        ("h", 3), ("w", 2), ("h", 4), ("h", 5), ("w", 3),
                        ("h", 6), ("h", 7)]
                for i, (kind, idx) in enumerate(plan):
                    eng = q3[i % 3]
                    if kind == "h":
                        eng.dma_start(
                            out=t0_[:, 2 * idx:2 * idx + 2, :],
                            in_=hsT4[:, 0, 2 * idx:2 * idx + 2, :])
                    else:
                        eng.dma_start(
                            out=wkv_sb[:, 4 * idx:4 * idx + 4, :],
                            in_=wkv_t[:, 4 * idx:4 * idx + 4, :])
                for dq in range(4):
                    eng = (nc.sync, nc.scalar, nc.gpsimd, nc.sync)[dq]
                    eng.dma_start(out=wq_sb[:, 4 * dq:4 * dq + 4, :],
                                  in_=wq_t[:, 4 * dq:4 * dq + 4, :])
                nc.gpsimd.dma_start(out=ck_sb[:], in_=ckT[:, :])
                nc.scalar.dma_start(out=sk_sb[:], in_=skT[:, :])
                nc.scalar.dma_start(out=cq_sb[:], in_=cqT[:, :])
                nc.gpsimd.dma_start(out=sq_sb[:], in_=sqT[:, :])
                fetch_hsT(1)
                nc.scalar.dma_start(out=wo_sb[:], in_=wo_t[:, :, :])

                # init constants (tiles from main pools: no address-space
                # WAR against the hsT prefetches)
                ident_f = st16_p.tile([P, P], f32, tag="x16")
                make_identity(nc, ident_f[:])
                nc.vector.tensor_copy(ident[:], ident_f[:])
                # corner keep-mask: maskc[kr, qc] = 1 where qc >= kr
                mask_f = st16_p.tile([P, P], f32, tag="x16")
                nc.gpsimd.memset(mask_f[:], 1.0)
                nc.gpsimd.affine_select(
                    out=mask_f[:], in_=mask_f[:],
                    compare_op=mybir.AluOpType.is_ge, fill=0.0,
                    base=0, pattern=[[1, P]], channel_multiplier=-1,
                )
                nc.vector.tensor_copy(maskc[:], mask_f[:])
                for st_i in range(ST):
                    for g in range(LKV):
                        nc.gpsimd.memset(
                            vaug[:, st_i, g * VW + HD:g * VW + HD + 1], 1.0)

                # project block 0 (k/q0/v first so attn(0,0) is ready asap)
                unit_k(0)
                unit_q(0, 0)
                unit_v(0)
                for c in range(1, FT):
                    unit_q(0, c)

                aT_tiles = {}
                for J in range(NJ):
                    # proj units for J+1 spread across this block's t-slots
                    units = []
                    if J + 1 < NJ:
                        jj = J + 1
                        units = [
                            [lambda: fetch_hsT(jj + 1), lambda: unit_k(jj),
                             lambda: unit_q(jj, 0)],
                            [lambda: unit_v(jj), lambda: unit_q(jj, 1)],
                            [lambda: unit_q(jj, 2)],
                            [lambda: unit_q(jj, 3)],
                        ]
                    aT_tiles[J] = aT_p.tile([P, FT, 512], f16, tag="aT",
                                            name=f"aT{J}")
                    for t in range(FT):
                        mf = None
                        if J >= 1:
                            prev_aT = aT_tiles[J - 1]
                            st_prev = 4 * (J - 1) + t
                            mf = (lambda a=prev_aT, s=st_prev:
                                  oproj_chunk(a, s))
                        attn_block(J, t, aT_tiles[J], midfill=mf,
                                   fine_norm=(J == NJ - 1 and t == FT - 1))
                        if units:
                            for u in units[t]:
                                u()
                    if J >= 2:
                        del aT_tiles[J - 2]
                for t in range(FT):
                    oproj_chunk(aT_tiles[NJ - 1], 4 * (NJ - 1) + t,
                                tail=True)
    nc.compile()
    return nc


def _host_tables(cos, sin, scale):
    # transposed [feature, S] tables, head-pair replicated (2 x 64 rows),
    # rotate-half sign folded into sin rows 0:32 of each 64-row head
    hd = cos.shape[1]                      # 64
    cosT = np.ascontiguousarray(cos.T)     # [64, S]
    sinT = np.ascontiguousarray(sin.T)
    sin_pm = np.concatenate([-sinT[:hd // 2], sinT[hd // 2:]], axis=0)
    cqT = np.tile(cosT * scale, (2, 1)).astype(np.float16)
    sqT = np.tile(sin_pm * scale, (2, 1)).astype(np.float16)
    ckT = np.tile(cosT, (2, 1)).astype(np.float16)
    skT = np.tile(sin_pm, (2, 1)).astype(np.float16)
    return cqT, sqT, ckT, skT


def prepare_in_maps(hidden_states, cos, sin, Wq, Wk, Wv, Wo, LQ=8, LKV=2):
    cos = np.asarray(cos, dtype=np.float32)
    sin = np.asarray(sin, dtype=np.float32)
    cqT, sqT, ckT, skT = _host_tables(cos, sin, SCALE)
    hidden_states = np.asarray(hidden_states, dtype=np.float32)
    Wq = np.asarray(Wq, dtype=np.float32)
    Wk = np.asarray(Wk, dtype=np.float32)
    Wv = np.asarray(Wv, dtype=np.float32)
    Wo = np.asarray(Wo, dtype=np.float32)
    nb = hidden_states.shape[0]
    DT, NJ, FT = 16, 4, 4
    # hsT4[p, j, dt, c] = hs.T[dt*128+p, j*512+c]
    hsT4 = [np.ascontiguousarray(
        hidden_states[b].T.reshape(DT, P, NJ, 512).transpose(1, 2, 0, 3)
    ).astype(np.float16) for b in range(nb)]
    in_maps = []
    for c in range(8):
        b, g2 = c // 4, c % 4
        qs = g2 * LQ * HD
        ks = g2 * LKV * HD
        wq_l = Wq[:, qs:qs + LQ * HD]
        wkv_l = np.concatenate([Wk[:, ks:ks + LKV * HD],
                                Wv[:, ks:ks + LKV * HD]], axis=1)
        wo_l = Wo[qs:qs + LQ * HD, :]
        in_maps.append({
            "hsT4": hsT4[b],
            "wqt": np.ascontiguousarray(
                wq_l.reshape(DT, P, LQ * HD).transpose(1, 0, 2)).astype(np.float16),
            "wkvt": np.ascontiguousarray(
                wkv_l.reshape(DT, P, 2 * LKV * HD).transpose(1, 0, 2)).astype(np.float16),
            "wot": np.ascontiguousarray(
                wo_l.reshape(FT, P, Wo.shape[1]).transpose(1, 0, 2)).astype(np.float16),
            "cqT": cqT, "sqT": sqT, "ckT": ckT, "skT": skT,
        })
    return in_maps


_NC_CACHE = {}


def kernel(hidden_states, attention_mask, cos, sin, Wq, Wk, Wv, Wo):
    from concourse.bass_utils import run_bass_kernel_spmd

    LQ, LKV = 8, 2
    if "nc" not in _NC_CACHE:
        _NC_CACHE["nc"] = build_nc(S, D, LQ, LKV, HD)
    nc = _NC_CACHE["nc"]

    in_maps = prepare_in_maps(hidden_states, cos, sin, Wq, Wk, Wv, Wo, LQ, LKV)
    res = run_bass_kernel_spmd(nc, in_maps, core_ids=list(range(8)))
    y = np.zeros((B, S, D), dtype=np.float32)
    for c in range(8):
        y[c // 4] += res.results[c]["out"].astype(np.float32)
    return y
